# revision 31
# baseline (speedup 1.0000x reference)
"""Trainium2 Bass kernel: Whisper-style self-attention (B=4, S=1500, D=1280, H=20).

Sharding: core c = 2*b + g handles batch b (of 4) and head-group g (of 2,
10 heads each).  Every matmul is exactly 1/8 of the total work:
  - Q/K/V projections column-sharded over the head group,
  - attention sharded by (batch, head),
  - output projection row-sharded; the two head-group partials of each batch
    are summed on the host (plus bias terms, which fold into host math).

Device dataflow (per core), all fp16 operands (PSUM f32):
  xT [1280,1500] -> qT,kT [640,1500] fp16 (qT scaled 1/8 + bq),
  v [1500,10,65] (64 v cols + ones col per head -> softmax Z).
  Per (head h, sq chunk c): scoresT = kT.T@qT per (128-row k tile), Exp
  batched over psum bank pairs on ACT -> expT fp16.  Then per 128-col sq
  subtile: ctx[sq,65] accumulated in PSUM with ex as the STATIONARY operand
  (12 matmuls of only 65 moving cols each - 2x fewer PE cycles than
  streaming expT), DVE reciprocal of the Z column + per-partition
  tensor_scalar multiply -> ctx_sb fp16 [sq,128] (head pair), then a DMA
  transpose (xbar) writes ctxT [128,sq] directly - no PE transpose, no
  PSUM->SBUF copy.  O-proj fp16 (ctxT stationary, wo moving).

Scheduling: sequencers are in-order and sem waits hold the SEQ, so the
emission order IS the schedule.  Attention units (software-pipelined one
unit deep) are ACT-bound: per 512-col chunk ACT needs ~6.2us of exp while
its scores+attnV PE work is only ~3.9us, and the ps2 WAR (bufs=2) stalls
PE inside the scores loop ~0.6us per psum pair.  So ALL other PE work
(projections, O-proj) is chopped into one-PSUM-group "micro" pieces and a
credit scheduler pumps ~one micro per scores pair, subject to
read-after-write deadlines (dl) and transpose-gating (nb).  ctx transposes
own the SP DMA queue; out stores ride Pool/SWDGE; weight loads are split
per 128-col slice so the first matmuls start ~2us after launch.
"""
import sys
sys.path.insert(0, "/opt/trn_rl_repo")

from collections import deque
from contextlib import ExitStack
import numpy as np

import concourse.bass as bass
import concourse.tile as tile
from concourse import bacc, mybir
from concourse.bass_utils import run_bass_kernel_spmd

dt = mybir.dt
AF = mybir.ActivationFunctionType
ALU = mybir.AluOpType

N_CORES = 8
B, S, D = 4, 1500, 1280
H, DH = 20, 64
G = 2
DG = D // G           # 640
HPG = H // G          # 10
KD = D // 128         # 10
MD = DG // 128        # 5
CW = (512, 512, 476)  # sq/proj chunk widths (PSUM-bank bound)
CO = (0, 512, 1024)   # chunk offsets
NS = 3
KS = (S + 127) // 128  # 12 (11*128 + 92)
ON = (512, 512, 256)
OO = (0, 512, 1024)
SP = S + 4            # ctxT padded to 1504 so the last 96-wide sq subtile
                      # (92 real cols) can be DMA-transposed whole

_CACHE = {}


def _sk(i):
    return min(128, S - i * 128)


def _subtiles(c):
    """(local_off, width) 128-col subtiles of chunk c; last one padded to 96."""
    w = CW[c]
    out = []
    off = 0
    while off < w:
        sw = min(128, w - off)
        if sw % 16:
            sw = 96  # pad 92 -> 96 for the xbar transpose (junk cols unread)
        out.append((off, sw))
        off += 128
    return out


def build():
    nc = bacc.Bacc("TRN2", target_bir_lowering=False, debug=False,
                   num_devices=N_CORES)
    xt_d = nc.dram_tensor("xt", [D, S], dt.float16, kind="ExternalInput").ap()
    # weights arrive pre-tiled [m, p, kk, j] so each 128-col slice is one
    # contiguous 2.5KB/partition DMA (256B rows would pay the 2x small-
    # element DMA penalty)
    wq_d = nc.dram_tensor("wq", [MD, 128, KD, 128], dt.float16,
                          kind="ExternalInput").ap()
    wk_d = nc.dram_tensor("wk", [MD, 128, KD, 128], dt.float16,
                          kind="ExternalInput").ap()
    wv_d = nc.dram_tensor("wv", [MD, 128, KD, 128], dt.float16,
                          kind="ExternalInput").ap()
    wo_d = nc.dram_tensor("wo", [DG, D], dt.float16, kind="ExternalInput").ap()
    bq_d = nc.dram_tensor("bq", [128, MD], dt.float32, kind="ExternalInput").ap()
    out_d = nc.dram_tensor("out", [S, D], dt.float16, kind="ExternalOutput").ap()

    xt_r = xt_d.rearrange("(k p) s -> p k s", p=128)
    wo_r = wo_d.rearrange("(k p) n -> p k n", p=128)

    with tile.TileContext(nc) as tc, ExitStack() as octx:
        persist = octx.enter_context(tc.tile_pool(name="persist", bufs=1))
        epool = octx.enter_context(tc.tile_pool(name="expT", bufs=3))
        zpool = octx.enter_context(tc.tile_pool(name="z", bufs=3))
        cpool = octx.enter_context(tc.tile_pool(name="ctxsb", bufs=12))
        opool = octx.enter_context(tc.tile_pool(name="ob", bufs=3))
        ps2 = octx.enter_context(tc.tile_pool(name="ps2", bufs=2, space="PSUM"))
        ps1 = octx.enter_context(tc.tile_pool(name="ps1", bufs=2, space="PSUM"))
        pat = octx.enter_context(tc.tile_pool(name="pat", bufs=2, space="PSUM"))

        qT = persist.tile([128, MD, S], dt.float16, tag="qT")
        kT = persist.tile([128, MD, S], dt.float16, tag="kT")
        v = persist.tile([128, KS, HPG, DH + 1], dt.float16, tag="v")
        ctxT = persist.tile([128, MD, SP], dt.float16, tag="ctxT")
        bq_s = persist.tile([128, MD], dt.float32, tag="bq")
        xt_s = persist.tile([128, KD, S], dt.float16, tag="xt")
        # m-major so each [128, KD, 128] m-slice is contiguous per partition
        # (2.5KB rows; 256B strided rows would pay the 2x small-elem DMA tax)
        wqs = persist.tile([128, MD, KD, 128], dt.float16, tag="wqs")
        wks = persist.tile([128, MD, KD, 128], dt.float16, tag="wks")
        wvs = persist.tile([128, MD, KD, 128], dt.float16, tag="wvs")
        wo_s = persist.tile([128, MD, D], dt.float16, tag="wo")

        # --- input DMAs, ordered for earliest first matmul (the cost model
        # serializes transfers on one DMA_ENGINES slot, so order matters):
        # the prelude computes kT c0, qT c0, v[h0-1] ms0-3, kT c1, v ms4-7,
        # kT c2 -- each group's data lands just before PE reaches it.
        nc.sync.dma_start(out=wks[:, 0].rearrange("p a b -> p (a b)"),
                          in_=wk_d[0].rearrange("p a b -> p (a b)"))
        for k2 in range(0, KD, 5):  # chunk 0 in halves: descgen serializes
            nc.sync.dma_start(out=xt_s[:, k2:k2 + 5, 0:CW[0]],
                              in_=xt_r[:, k2:k2 + 5, 0:CW[0]])
        nc.sync.dma_start(out=wqs[:, 0].rearrange("p a b -> p (a b)"),
                          in_=wq_d[0].rearrange("p a b -> p (a b)"))
        nc.sync.dma_start(out=wvs[:, 0].rearrange("p a b -> p (a b)"),
                          in_=wv_d[0].rearrange("p a b -> p (a b)"))
        nc.sync.dma_start(out=bq_s[:], in_=bq_d[:])
        nc.sync.dma_start(out=xt_s[:, :, CO[1]:CO[1] + CW[1]],
                          in_=xt_r[:, :, CO[1]:CO[1] + CW[1]])
        nc.sync.dma_start(out=xt_s[:, :, CO[2]:CO[2] + CW[2]],
                          in_=xt_r[:, :, CO[2]:CO[2] + CW[2]])
        for m in range(1, MD):
            for ws, wd in ((wks, wk_d), (wqs, wq_d), (wvs, wv_d)):
                nc.sync.dma_start(
                    out=ws[:, m].rearrange("p a b -> p (a b)"),
                    in_=wd[m].rearrange("p a b -> p (a b)"))

        ones1 = persist.tile([128, 1], dt.float16, tag="ones1")
        nc.vector.memset(ones1[:], 1.0)
        nc.vector.tensor_copy(v[:, :, :, DH:DH + 1],
                              ones1[:].to_broadcast([128, KS, HPG, 1]))

        # ---- micro building blocks (one ps1 PSUM group each) -----------
        def qk_micro(m, which, n):
            """One sq chunk of the q or k projection for d-tile m (~2.1us)."""
            ws = wqs if which == "q" else wks
            cw, co = CW[n], CO[n]
            ps = ps1.tile([128, 1, 512], dt.float32, tag="ps1", name="ps1")
            for kk in range(KD):
                nc.tensor.matmul(
                    ps[:, 0, 0:cw],
                    lhsT=ws[:, m, kk, :],
                    rhs=xt_s[:, kk, co:co + cw],
                    start=(kk == 0), stop=(kk == KD - 1))
            if which == "q":
                nc.vector.tensor_scalar(
                    qT[:, m, co:co + cw], ps[:, 0, 0:cw], 0.125,
                    bq_s[:, m:m + 1], op0=ALU.mult, op1=ALU.add)
            else:
                nc.vector.tensor_copy(kT[:, m, co:co + cw], ps[:, 0, 0:cw])

        def v_micro(hp, ms):
            """v columns for head pair hp, one 128-row s tile (~0.6us)."""
            sp = _sk(ms)
            ps = ps1.tile([128, 1, 512], dt.float32, tag="ps1", name="ps1")
            for kk in range(KD):
                nc.tensor.matmul(
                    ps[0:sp, 0, 0:128],
                    lhsT=xt_s[:, kk, ms * 128:ms * 128 + sp],
                    rhs=wvs[:, hp, kk, :],
                    start=(kk == 0), stop=(kk == KD - 1))
            nc.vector.tensor_copy(
                v[0:sp, ms, 2 * hp:2 * hp + 2, 0:DH],
                ps[0:sp, 0, 0:128].rearrange("p (h e) -> p h e", h=2))

        def wo_micro():
            nc.sync.dma_start(out=wo_s[:], in_=wo_r[:])

        def op_micro(ms, j):
            """One 512-col group of the O-projection for sq tile ms."""
            sp = _sk(ms)
            nw, noff = ON[j], OO[j]
            ps = ps1.tile([128, 1, 512], dt.float32, tag="ps1", name="ps1")
            for kk in range(MD):
                nc.tensor.matmul(
                    ps[0:sp, 0, 0:nw],
                    lhsT=ctxT[:, kk, ms * 128:ms * 128 + sp],
                    rhs=wo_s[:, kk, noff:noff + nw],
                    start=(kk == 0), stop=(kk == MD - 1))
            ob = opool.tile([128, 512], dt.float16, tag="ob", name="ob")
            nc.vector.tensor_copy(ob[0:sp, 0:nw], ps[0:sp, 0, 0:nw])
            # Mid-kernel out-stores ride Pool/SWDGE (SP.SEQ is busy with ctx
            # transposes whose sem waits hold it); the final sq tiles
            # alternate queues so the drain overlaps.
            eng = nc.sync if (ms >= 8 and j >= 1) else nc.gpsimd
            eng.dma_start(
                out=out_d[ms * 128:ms * 128 + sp, noff:noff + nw],
                in_=ob[0:sp, 0:nw])

        # ---- attention unit pieces -------------------------------------
        def emit_scores(h, c, pump):
            base = 64 * (h % 2)
            td = h // 2
            cw, co = CW[c], CO[c]
            csl = slice(co, co + cw)
            ex = epool.tile([128, KS, 512], dt.float16, tag="expT", name="ex")
            for kk2 in range(0, KS, 2):
                ps = ps2.tile([128, 2, 512], dt.float32, tag="ps2", name="ps2")
                for j in range(2):
                    kk = kk2 + j
                    sp = _sk(kk)
                    nc.tensor.matmul(
                        ps[0:sp, j, 0:cw],
                        lhsT=kT[base:base + 64, td, kk * 128:kk * 128 + sp],
                        rhs=qT[base:base + 64, td, csl],
                        start=True, stop=True)
                nc.scalar.activation(ex[:, kk2:kk2 + 2, 0:cw], ps[:, :, 0:cw],
                                     AF.Exp)
                pump()
            return ex

        csb_live = {}

        def emit_tail(h, c, ex, after_subtile=None):
            """attnV (ex stationary) + 1/Z scale into the pair's ctx_sb.
            For odd h the subtile's transpose is emitted as soon as both
            halves are written; after_subtile(t_idx) can interleave extra PE
            work (used to overlap the final O-proj with the last tail)."""
            td, hb = h // 2, 64 * (h % 2)
            if (td, c) not in csb_live:
                csb_live[(td, c)] = {
                    off: cpool.tile([128, 128], dt.float16, tag="ctxsb",
                                    name="ctxsb")
                    for off, _ in _subtiles(c)}
            csb = csb_live[(td, c)]
            for ti, (off, sw) in enumerate(_subtiles(c)):
                pc = pat.tile([128, DH + 1], dt.float32, tag="pat", name="pat")
                for kk in range(KS):
                    sp = _sk(kk)
                    nc.tensor.matmul(
                        pc[0:sw, :],
                        lhsT=ex[0:sp, kk, off:off + sw],
                        rhs=v[0:sp, kk, h, :],
                        start=(kk == 0), stop=(kk == KS - 1))
                rz = zpool.tile([128, 1], dt.float32, tag="rz", name="rz")
                nc.vector.reciprocal(rz[0:sw, :], pc[0:sw, DH:DH + 1])
                nc.vector.tensor_scalar(
                    csb[off][0:sw, hb:hb + 64], pc[0:sw, 0:DH], rz[0:sw, :],
                    None, op0=ALU.mult)
                if h % 2 == 1:
                    nc.sync.dma_start(
                        out=ctxT[:, td, CO[c] + off:CO[c] + off + sw],
                        in_=csb[off][0:sw, :], transpose=True)
                    if after_subtile is not None:
                        after_subtile(ti)
            if h % 2 == 1:
                del csb_live[(td, c)]

        # ---- schedule --------------------------------------------------
        # c-major pair order: all c0 pairs first => O-proj for sq<512 can
        # start as filler at iteration 11, sq<1024 at 21.
        pairs = [(td, c) for c in (0, 1, 2) for td in range(5)]
        units = [(2 * td + o, c) for td, c in pairs for o in (0, 1)]

        # micro list: (cost_rows, dl, nb, fn); consumed strictly in order.
        # dl: must be emitted before scores of that iteration (RAW via
        # emission order).  nb: not before that iteration (transpose gating).
        M = []
        for m in range(1, MD):
            for n in range(NS):
                M.append((10 * CW[n], 2 * m, 0,
                          lambda m=m, n=n: qk_micro(m, "k", n)))
            M.append((10 * CW[0], 2 * m, 0, lambda m=m: qk_micro(m, "q", 0)))
            for ms in range(KS):
                M.append((1280, 2 * m + 1, 0,
                          lambda m=m, ms=ms: v_micro(m, ms)))
        M.append((0, 9, 0, wo_micro))
        # O-proj micros carry staggered deadlines so forced drains spread
        # them across the otherwise-dry c1/c2 regions.
        op_dl = {0: (12, 12, 13), 1: (13, 14, 14), 2: (15, 15, 16),
                 3: (16, 17, 17), 4: (21, 21, 22), 5: (22, 23, 23),
                 6: (24, 24, 25), 7: (25, 26, 26)}
        for m in range(0, MD):
            M.append((10 * CW[1], 10 + 2 * m, max(0, 6 + 2 * m),
                      lambda m=m: qk_micro(m, "q", 1)))
            if m in (1, 2):
                for ms in (2 * m - 2, 2 * m - 1):
                    for j in range(NS):
                        M.append((5 * ON[j], op_dl[ms][j], 11,
                                  lambda ms=ms, j=j: op_micro(ms, j)))
        for m in range(0, MD):
            M.append((10 * CW[2], 20 + 2 * m, 16 + 2 * m,
                      lambda m=m: qk_micro(m, "q", 2)))
            if m in (1, 2):
                for ms in (2 * m + 2, 2 * m + 3):
                    for j in range(NS):
                        M.append((5 * ON[j], op_dl[ms][j], 21,
                                  lambda ms=ms, j=j: op_micro(ms, j)))
        mq = deque(M)
        # pace matches the per-pair ACT deficit (~930 PE rows): ACT needs
        # ~6.2us/unit of exp vs ~3.9us of scores+attnV PE work.  Pumping
        # faster than the deficit just drains the queue early and leaves
        # the late units dry; deadline drains place the surplus.
        pace = 930.0

        state = {"iter": 0, "debt": 0.0}

        def drain_deadlines():
            # pop through the LAST due micro (due ones may sit behind
            # not-yet-due ops in the strictly-ordered queue)
            it = state["iter"]
            idx = -1
            for k, m in enumerate(mq):
                if m[1] <= it:
                    idx = k
            for _ in range(idx + 1):
                _, _, nb, fn = mq.popleft()
                assert nb <= it, "nb violation forced by a deadline"
                fn()

        def pump():
            state["debt"] += pace
            while mq and state["debt"] > 0 and mq[0][2] <= state["iter"]:
                cost, _, _, fn = mq.popleft()
                fn()
                state["debt"] -= cost

        # prelude: only kT/qT d-tile 0 chunk 0 -- unit 0's scores pairs are
        # then interleaved with the REST of the prelude (kT c1/c2, v[h0-1])
        # so attention starts ~6us earlier and the later kT chunk groups
        # hide the xt c1/c2 DMA waits behind ready scores/v work.
        qk_micro(0, "k", 0)
        qk_micro(0, "q", 0)
        p0seq = {
            1: [lambda: [v_micro(0, ms) for ms in range(0, 4)],
                lambda: qk_micro(0, "k", 1)],
            3: [lambda: [v_micro(0, ms) for ms in range(4, 8)],
                lambda: qk_micro(0, "k", 2)],
            5: [lambda: [v_micro(0, ms) for ms in range(8, KS)]],
        }
        p0 = {"j": 0}

        def pump0():
            j = p0["j"]
            p0["j"] += 1
            for f in p0seq.get(j, []):
                f()

        exm = {}
        for i, u in enumerate(units):
            state["iter"] = i
            drain_deadlines()
            exm[u] = emit_scores(u[0], u[1], pump0 if i == 0 else pump)
            if i >= 1:
                up = units[i - 1]
                emit_tail(up[0], up[1], exm.pop(up))
        up = units[-1]
        state["iter"] = len(units)
        while mq:
            _, _, _, fn = mq.popleft()
            fn()

        def tail_hook(ti):
            # overlap the final O-proj with the last tail: two subtiles
            # after a transpose, its O-proj runs (the in-between attnV +
            # O-proj work hides the transpose DMA latency).
            if ti >= 2:
                for j in range(NS):
                    op_micro(8 + ti - 2, j)
        emit_tail(up[0], up[1], exm.pop(up), after_subtile=tail_hook)
        for ms in (10, 11):
            for j in range(NS):
                op_micro(ms, j)

    nc.compile()
    return nc


def _get_nc():
    if "nc" not in _CACHE:
        _CACHE["nc"] = build()
    return _CACHE["nc"]


def _tile_w(WT):
    """[1280, 640] K-major weight -> [m, p, kk, j] so each m-slice DMA is
    one contiguous 2.5KB/partition read."""
    a = WT.reshape(KD, 128, MD, 128)
    return np.ascontiguousarray(a.transpose(2, 1, 0, 3))


def _prep_in_maps(x, Wq, bq, Wk, Wv, Wo):
    in_maps = []
    for c in range(N_CORES):
        b, g = divmod(c, G)
        gs = slice(g * DG, (g + 1) * DG)
        in_maps.append({
            "xt": np.ascontiguousarray(x[b].T).astype(np.float16),
            "wq": _tile_w(Wq[gs, :].T.astype(np.float16)),
            "wk": _tile_w(Wk[gs, :].T.astype(np.float16)),
            "wv": _tile_w(Wv[gs, :].T.astype(np.float16)),
            "wo": np.ascontiguousarray(Wo[:, gs].T).astype(np.float16),
            "bq": np.ascontiguousarray(
                (0.125 * bq[gs]).astype(np.float32).reshape(MD, 128).T),
        })
    return in_maps


def run(x, Wq, bq, Wk, Wv, bv, Wo, bo, trace=False, **trace_kw):
    x = np.asarray(x, dtype=np.float32)
    Wq = np.asarray(Wq, dtype=np.float32)
    bq = np.asarray(bq, dtype=np.float32)
    Wk = np.asarray(Wk, dtype=np.float32)
    Wv = np.asarray(Wv, dtype=np.float32)
    bv = np.asarray(bv, dtype=np.float32)
    Wo = np.asarray(Wo, dtype=np.float32)
    bo = np.asarray(bo, dtype=np.float32)

    nc = _get_nc()
    in_maps = _prep_in_maps(x, Wq, bq, Wk, Wv, Wo)
    res = None
    for attempt in range(3):
        try:
            res = run_bass_kernel_spmd(nc, in_maps, list(range(N_CORES)),
                                       trace=trace, **trace_kw)
            break
        except Exception:
            # Sporadic NRT_EXEC_UNIT_UNRECOVERABLE on first exec; devices
            # come back after ~75s. Reset the backend and retry.
            if attempt == 2:
                raise
            import time as _time
            import jax as _jax
            _time.sleep(80)
            try:
                _jax.clear_backends()
            except Exception:
                pass
    const = (bv @ Wo.T + bo).astype(np.float32)  # [D]
    out = np.empty((B, S, D), dtype=np.float32)
    for b in range(B):
        out[b] = (res.results[2 * b]["out"].astype(np.float32)
                  + res.results[2 * b + 1]["out"].astype(np.float32) + const)
    return out, res


def kernel(**inputs):
    out, _ = run(**inputs)
    return out


# revision 34
# speedup vs baseline: 1.0687x; 1.0687x over previous
"""Trainium2 Bass kernel: Whisper-style self-attention (B=4, S=1500, D=1280, H=20).

Sharding: core c = 2*b + g handles batch b (of 4) and head-group g (of 2,
10 heads each).  Every matmul is exactly 1/8 of the total work:
  - Q/K/V projections column-sharded over the head group,
  - attention sharded by (batch, head),
  - output projection row-sharded; the two head-group partials of each batch
    are summed on the host (plus bias terms, which fold into host math).

Device dataflow (per core), all fp16 operands (PSUM f32):
  xT [1280,1500] -> qT,kT [640,1500] fp16 (qT scaled 1/8 + bq),
  v [1500,10,65] (64 v cols + ones col per head -> softmax Z).
  Per (head h, sq chunk c): scoresT = kT.T@qT per (128-row k tile), Exp
  batched over psum bank pairs on ACT -> expT fp16.  Then per 128-col sq
  subtile: ctx[sq,65] accumulated in PSUM with ex as the STATIONARY operand
  (12 matmuls of only 65 moving cols each - 2x fewer PE cycles than
  streaming expT), DVE reciprocal of the Z column + per-partition
  tensor_scalar multiply -> ctx_sb fp16 [sq,128] (head pair), then a DMA
  transpose (xbar) writes ctxT [128,sq] directly - no PE transpose, no
  PSUM->SBUF copy.  O-proj fp16 (ctxT stationary, wo moving).

Scheduling: sequencers are in-order and sem waits hold the SEQ, so the
emission order IS the schedule.  Attention units (software-pipelined one
unit deep) are ACT-bound: per 512-col chunk ACT needs ~6.2us of exp while
its scores+attnV PE work is only ~3.9us, and the ps2 WAR (bufs=2) stalls
PE inside the scores loop ~0.6us per psum pair.  So ALL other PE work
(projections, O-proj) is chopped into one-PSUM-group "micro" pieces and a
credit scheduler pumps ~one micro per scores pair, subject to
read-after-write deadlines (dl) and transpose-gating (nb).  ctx transposes
own the SP DMA queue; out stores ride Pool/SWDGE; weight loads are split
per 128-col slice so the first matmuls start ~2us after launch.
"""
import sys
sys.path.insert(0, "/opt/trn_rl_repo")

from collections import deque
from contextlib import ExitStack
import numpy as np

import concourse.bass as bass
import concourse.tile as tile
from concourse import bacc, mybir
from concourse.bass_utils import run_bass_kernel_spmd

dt = mybir.dt
AF = mybir.ActivationFunctionType
ALU = mybir.AluOpType

N_CORES = 8
B, S, D = 4, 1500, 1280
H, DH = 20, 64
G = 2
DG = D // G           # 640
HPG = H // G          # 10
KD = D // 128         # 10
MD = DG // 128        # 5
CW = (512, 512, 476)  # sq/proj chunk widths (PSUM-bank bound)
CO = (0, 512, 1024)   # chunk offsets
NS = 3
KS = (S + 127) // 128  # 12 (11*128 + 92)
ON = (512, 512, 256)
OO = (0, 512, 1024)
SP = S + 4            # ctxT padded to 1504 so the last 96-wide sq subtile
                      # (92 real cols) can be DMA-transposed whole

_CACHE = {}


def _sk(i):
    return min(128, S - i * 128)


def _subtiles(c):
    """(local_off, width) 128-col subtiles of chunk c; last one padded to 96."""
    w = CW[c]
    out = []
    off = 0
    while off < w:
        sw = min(128, w - off)
        if sw % 16:
            sw = 96  # pad 92 -> 96 for the xbar transpose (junk cols unread)
        out.append((off, sw))
        off += 128
    return out


def build():
    nc = bacc.Bacc("TRN2", target_bir_lowering=False, debug=False,
                   num_devices=N_CORES)
    # x and the QKV weights arrive as fp8e4m3 hi/lo pairs (W pre-scaled by
    # 32 so the lo residual stays out of fp8's subnormal range; descale is
    # folded into the psum->SBUF copy).  The projections then run DoubleRow
    # fp8 matmuls: K=256 per instruction at 0.5 cycles/row -> 4x the fp16
    # GEMM rate, 3 hi/lo terms (xh*wh + xh*wl + xl*wh) -> net 25% fewer PE
    # rows with ~0.1% relative error.  Layouts are pre-tiled host-side so
    # every DMA is a contiguous >=512B-row read.
    # x is staged s-tile-major [p, ms, kp, two, j] (cols zero-padded to
    # 1536): the v projection needs x STATIONARY with pair-adjacent blocks
    # (the ldweights ISA rejects a 1500-stride pair dim), and the qk
    # projections read the same tile as a nested moving AP.
    xh_d = nc.dram_tensor("xh", [128, KS, KD // 2, 2, 128], dt.float8e4,
                          kind="ExternalInput").ap()
    xl_d = nc.dram_tensor("xl", [128, KS, KD // 2, 2, 128], dt.float8e4,
                          kind="ExternalInput").ap()
    w8_d = {}
    for t8 in ("qh", "ql", "kh", "kl", "vh", "vl"):
        w8_d[t8] = nc.dram_tensor(
            "w" + t8, [MD, 128, KD // 2, 2, 128], dt.float8e4,
            kind="ExternalInput").ap()
    wo_d = nc.dram_tensor("wo", [DG, D], dt.float16, kind="ExternalInput").ap()
    bq_d = nc.dram_tensor("bq", [128, MD], dt.float32, kind="ExternalInput").ap()
    out_d = nc.dram_tensor("out", [S, D], dt.float16, kind="ExternalOutput").ap()

    wo_r = wo_d.rearrange("(k p) n -> p k n", p=128)
    KP = KD // 2  # 5 DoubleRow k-pair groups

    with tile.TileContext(nc) as tc, ExitStack() as octx:
        persist = octx.enter_context(tc.tile_pool(name="persist", bufs=1))
        epool = octx.enter_context(tc.tile_pool(name="expT", bufs=3))
        zpool = octx.enter_context(tc.tile_pool(name="z", bufs=3))
        cpool = octx.enter_context(tc.tile_pool(name="ctxsb", bufs=12))
        opool = octx.enter_context(tc.tile_pool(name="ob", bufs=3))
        ps2 = octx.enter_context(tc.tile_pool(name="ps2", bufs=2, space="PSUM"))
        ps1 = octx.enter_context(tc.tile_pool(name="ps1", bufs=2, space="PSUM"))
        pat = octx.enter_context(tc.tile_pool(name="pat", bufs=2, space="PSUM"))

        qT = persist.tile([128, MD, KS * 128], dt.float16, tag="qT")
        kT = persist.tile([128, MD, KS * 128], dt.float16, tag="kT")
        v = persist.tile([128, KS, HPG, DH + 1], dt.float16, tag="v")
        ctxT = persist.tile([128, MD, SP], dt.float16, tag="ctxT")
        bq_s = persist.tile([128, MD], dt.float32, tag="bq")
        xh_s = persist.tile([128, KS, KP, 2, 128], dt.float8e4, tag="xh")
        xl_s = persist.tile([128, KS, KP, 2, 128], dt.float8e4, tag="xl")
        w8 = {t8: persist.tile([128, MD, KP, 2, 128], dt.float8e4,
                               tag="w" + t8, name="w8")
              for t8 in ("qh", "ql", "kh", "kl", "vh", "vl")}
        wo_s = persist.tile([128, MD, D], dt.float16, tag="wo")

        # --- input DMAs, ordered for earliest first matmul (the cost model
        # serializes transfers on one DMA_ENGINES slot, so order matters):
        # the prelude computes kT c0, qT c0, v[h0-1] ms0-3, kT c1, v ms4-7,
        # kT c2 -- each group's data lands just before PE reaches it.
        def wdma(t8, m):
            nc.sync.dma_start(
                out=w8[t8][:, m].rearrange("p a b c -> p (a b c)"),
                in_=w8_d[t8][m].rearrange("p a b c -> p (a b c)"))

        def xdma(xs_s, xs_d, n):
            nc.sync.dma_start(
                out=xs_s[:, 4 * n:4 * n + 4].rearrange(
                    "p a b c d -> p (a b c d)"),
                in_=xs_d[:, 4 * n:4 * n + 4].rearrange(
                    "p a b c d -> p (a b c d)"))

        wdma("kh", 0)
        xdma(xh_s, xh_d, 0)
        wdma("kl", 0)
        xdma(xl_s, xl_d, 0)
        wdma("qh", 0)
        wdma("ql", 0)
        nc.sync.dma_start(out=bq_s[:], in_=bq_d[:])
        xdma(xh_s, xh_d, 1)
        xdma(xl_s, xl_d, 1)
        wdma("vh", 0)
        wdma("vl", 0)
        xdma(xh_s, xh_d, 2)
        xdma(xl_s, xl_d, 2)
        for m in range(1, MD):
            for t8 in ("kh", "kl", "qh", "ql", "vh", "vl"):
                wdma(t8, m)

        ones1 = persist.tile([128, 1], dt.float16, tag="ones1")
        nc.vector.memset(ones1[:], 1.0)
        nc.vector.tensor_copy(v[:, :, :, DH:DH + 1],
                              ones1[:].to_broadcast([128, KS, HPG, 1]))

        # ---- micro building blocks (one ps1 PSUM group each) -----------
        DR = mybir.MatmulPerfMode.DoubleRow
        TERMS = lambda t: ((xh_s, w8[t + "h"]), (xh_s, w8[t + "l"]),
                           (xl_s, w8[t + "h"]))

        def qk_micro(m, which, n):
            """One 512-col sq chunk of the q or k projection for d-tile m
            (~1.6us): 3-term hi/lo fp8 DoubleRow, descale 1/32 in the psum
            copy.  Chunk c2 includes the zero-padded cols 1500-1535."""
            co = CO[n]
            ps = ps1.tile([128, 1, 512], dt.float32, tag="ps1", name="ps1")
            first, last = (0, 0), (2, KP - 1)
            for ti, (xs, wt) in enumerate(TERMS(which)):
                for kp in range(KP):
                    nc.tensor.matmul(
                        ps[:, 0, 0:512],
                        lhsT=wt[:, m, kp],
                        rhs=xs[:, 4 * n:4 * n + 4, kp].rearrange(
                            "p a b c -> p b a c"),
                        start=((ti, kp) == first), stop=((ti, kp) == last),
                        perf_mode=DR)
            if which == "q":
                nc.vector.tensor_scalar(
                    qT[:, m, co:co + 512], ps[:, 0, 0:512], 0.125 / 32.0,
                    bq_s[:, m:m + 1], op0=ALU.mult, op1=ALU.add)
            else:
                nc.vector.tensor_scalar(
                    kT[:, m, co:co + 512], ps[:, 0, 0:512], 1.0 / 32.0,
                    None, op0=ALU.mult)

        def v_micro(hp, ms):
            """v columns for head pair hp, one 128-row s tile (~0.4us)."""
            sp = _sk(ms)
            ps = ps1.tile([128, 1, 512], dt.float32, tag="ps1", name="ps1")
            first, last = (0, 0), (2, KP - 1)
            for ti, (xs, wt) in enumerate(TERMS("v")):
                for kp in range(KP):
                    nc.tensor.matmul(
                        ps[0:sp, 0, 0:128],
                        lhsT=xs[:, ms, kp, :, 0:sp],
                        rhs=wt[:, hp, kp],
                        start=((ti, kp) == first), stop=((ti, kp) == last),
                        perf_mode=DR)
            nc.vector.tensor_scalar(
                v[0:sp, ms, 2 * hp:2 * hp + 2, 0:DH],
                ps[0:sp, 0, 0:128].rearrange("p (h e) -> p h e", h=2),
                1.0 / 32.0, None, op0=ALU.mult)

        def wo_micro():
            nc.sync.dma_start(out=wo_s[:], in_=wo_r[:])

        def op_micro(ms, j):
            """One 512-col group of the O-projection for sq tile ms."""
            sp = _sk(ms)
            nw, noff = ON[j], OO[j]
            ps = ps1.tile([128, 1, 512], dt.float32, tag="ps1", name="ps1")
            for kk in range(MD):
                nc.tensor.matmul(
                    ps[0:sp, 0, 0:nw],
                    lhsT=ctxT[:, kk, ms * 128:ms * 128 + sp],
                    rhs=wo_s[:, kk, noff:noff + nw],
                    start=(kk == 0), stop=(kk == MD - 1))
            ob = opool.tile([128, 512], dt.float16, tag="ob", name="ob")
            nc.vector.tensor_copy(ob[0:sp, 0:nw], ps[0:sp, 0, 0:nw])
            # Mid-kernel out-stores ride Pool/SWDGE (SP.SEQ is busy with ctx
            # transposes whose sem waits hold it); the final sq tiles
            # alternate queues so the drain overlaps.
            eng = nc.sync if (ms >= 8 and j >= 1) else nc.gpsimd
            eng.dma_start(
                out=out_d[ms * 128:ms * 128 + sp, noff:noff + nw],
                in_=ob[0:sp, 0:nw])

        # ---- attention unit pieces -------------------------------------
        def emit_scores(h, c, pump):
            base = 64 * (h % 2)
            td = h // 2
            cw, co = CW[c], CO[c]
            csl = slice(co, co + cw)
            ex = epool.tile([128, KS, 512], dt.float16, tag="expT", name="ex")
            for kk2 in range(0, KS, 2):
                ps = ps2.tile([128, 2, 512], dt.float32, tag="ps2", name="ps2")
                for j in range(2):
                    kk = kk2 + j
                    sp = _sk(kk)
                    nc.tensor.matmul(
                        ps[0:sp, j, 0:cw],
                        lhsT=kT[base:base + 64, td, kk * 128:kk * 128 + sp],
                        rhs=qT[base:base + 64, td, csl],
                        start=True, stop=True)
                nc.scalar.activation(ex[:, kk2:kk2 + 2, 0:cw], ps[:, :, 0:cw],
                                     AF.Exp)
                pump()
            return ex

        csb_live = {}

        def emit_tail(h, c, ex, after_subtile=None):
            """attnV (ex stationary) + 1/Z scale into the pair's ctx_sb.
            For odd h the subtile's transpose is emitted as soon as both
            halves are written; after_subtile(t_idx) can interleave extra PE
            work (used to overlap the final O-proj with the last tail)."""
            td, hb = h // 2, 64 * (h % 2)
            if (td, c) not in csb_live:
                csb_live[(td, c)] = {
                    off: cpool.tile([128, 128], dt.float16, tag="ctxsb",
                                    name="ctxsb")
                    for off, _ in _subtiles(c)}
            csb = csb_live[(td, c)]
            for ti, (off, sw) in enumerate(_subtiles(c)):
                pc = pat.tile([128, DH + 1], dt.float32, tag="pat", name="pat")
                for kk in range(KS):
                    sp = _sk(kk)
                    nc.tensor.matmul(
                        pc[0:sw, :],
                        lhsT=ex[0:sp, kk, off:off + sw],
                        rhs=v[0:sp, kk, h, :],
                        start=(kk == 0), stop=(kk == KS - 1))
                rz = zpool.tile([128, 1], dt.float32, tag="rz", name="rz")
                nc.vector.reciprocal(rz[0:sw, :], pc[0:sw, DH:DH + 1])
                nc.vector.tensor_scalar(
                    csb[off][0:sw, hb:hb + 64], pc[0:sw, 0:DH], rz[0:sw, :],
                    None, op0=ALU.mult)
                if h % 2 == 1:
                    nc.sync.dma_start(
                        out=ctxT[:, td, CO[c] + off:CO[c] + off + sw],
                        in_=csb[off][0:sw, :], transpose=True)
                    if after_subtile is not None:
                        after_subtile(ti)
            if h % 2 == 1:
                del csb_live[(td, c)]

        # ---- schedule --------------------------------------------------
        # c-major pair order: all c0 pairs first => O-proj for sq<512 can
        # start as filler at iteration 11, sq<1024 at 21.
        pairs = [(td, c) for c in (0, 1, 2) for td in range(5)]
        units = [(2 * td + o, c) for td, c in pairs for o in (0, 1)]

        # micro list: (cost_rows, dl, nb, fn); consumed strictly in order.
        # dl: must be emitted before scores of that iteration (RAW via
        # emission order).  nb: not before that iteration (transpose gating).
        M = []
        for m in range(1, MD):
            for n in range(NS):
                M.append((int(7.5 * CW[n]), 2 * m, 0,
                          lambda m=m, n=n: qk_micro(m, "k", n)))
            M.append((int(7.5 * CW[0]), 2 * m, 0,
                      lambda m=m: qk_micro(m, "q", 0)))
            for ms in range(KS):
                M.append((960, 2 * m + 1, 0,
                          lambda m=m, ms=ms: v_micro(m, ms)))
        M.append((0, 9, 0, wo_micro))
        # O-proj micros carry staggered deadlines so forced drains spread
        # them across the otherwise-dry c1/c2 regions.
        op_dl = {0: (12, 12, 13), 1: (13, 14, 14), 2: (15, 15, 16),
                 3: (16, 17, 17), 4: (21, 21, 22), 5: (22, 23, 23),
                 6: (24, 24, 25), 7: (25, 26, 26)}
        for m in range(0, MD):
            M.append((int(7.5 * CW[1]), 10 + 2 * m, max(0, 6 + 2 * m),
                      lambda m=m: qk_micro(m, "q", 1)))
            if m in (1, 2):
                for ms in (2 * m - 2, 2 * m - 1):
                    for j in range(NS):
                        M.append((5 * ON[j], op_dl[ms][j], 11,
                                  lambda ms=ms, j=j: op_micro(ms, j)))
        for m in range(0, MD):
            M.append((int(7.5 * CW[2]), 20 + 2 * m, 16 + 2 * m,
                      lambda m=m: qk_micro(m, "q", 2)))
            if m in (1, 2):
                for ms in (2 * m + 2, 2 * m + 3):
                    for j in range(NS):
                        M.append((5 * ON[j], op_dl[ms][j], 21,
                                  lambda ms=ms, j=j: op_micro(ms, j)))
        mq = deque(M)
        # pace matches the per-pair ACT deficit (~930 PE rows): ACT needs
        # ~6.2us/unit of exp vs ~3.9us of scores+attnV PE work.  Pumping
        # faster than the deficit just drains the queue early and leaves
        # the late units dry; deadline drains place the surplus.
        pace = 930.0

        state = {"iter": 0, "debt": 0.0}

        def drain_deadlines():
            # pop through the LAST due micro (due ones may sit behind
            # not-yet-due ops in the strictly-ordered queue)
            it = state["iter"]
            idx = -1
            for k, m in enumerate(mq):
                if m[1] <= it:
                    idx = k
            for _ in range(idx + 1):
                _, _, nb, fn = mq.popleft()
                assert nb <= it, "nb violation forced by a deadline"
                fn()

        def pump():
            state["debt"] += pace
            while mq and state["debt"] > 0 and mq[0][2] <= state["iter"]:
                cost, _, _, fn = mq.popleft()
                fn()
                state["debt"] -= cost

        # prelude: only kT/qT d-tile 0 chunk 0 -- unit 0's scores pairs are
        # then interleaved with the REST of the prelude (kT c1/c2, v[h0-1])
        # so attention starts ~6us earlier and the later kT chunk groups
        # hide the xt c1/c2 DMA waits behind ready scores/v work.
        qk_micro(0, "k", 0)
        qk_micro(0, "q", 0)
        p0seq = {
            1: [lambda: [v_micro(0, ms) for ms in range(0, 4)],
                lambda: qk_micro(0, "k", 1)],
            3: [lambda: [v_micro(0, ms) for ms in range(4, 8)],
                lambda: qk_micro(0, "k", 2)],
            5: [lambda: [v_micro(0, ms) for ms in range(8, KS)]],
        }
        p0 = {"j": 0}

        def pump0():
            j = p0["j"]
            p0["j"] += 1
            for f in p0seq.get(j, []):
                f()

        exm = {}
        for i, u in enumerate(units):
            state["iter"] = i
            drain_deadlines()
            exm[u] = emit_scores(u[0], u[1], pump0 if i == 0 else pump)
            if i >= 1:
                up = units[i - 1]
                emit_tail(up[0], up[1], exm.pop(up))
        up = units[-1]
        state["iter"] = len(units)
        while mq:
            _, _, _, fn = mq.popleft()
            fn()

        def tail_hook(ti):
            # overlap the final O-proj with the last tail: two subtiles
            # after a transpose, its O-proj runs (the in-between attnV +
            # O-proj work hides the transpose DMA latency).
            if ti >= 2:
                for j in range(NS):
                    op_micro(8 + ti - 2, j)
        emit_tail(up[0], up[1], exm.pop(up), after_subtile=tail_hook)
        for ms in (10, 11):
            for j in range(NS):
                op_micro(ms, j)

    nc.compile()
    return nc


def _get_nc():
    if "nc" not in _CACHE:
        _CACHE["nc"] = build()
    return _CACHE["nc"]


F8 = None


def _f8():
    global F8
    if F8 is None:
        import ml_dtypes
        F8 = ml_dtypes.float8_e4m3
    return F8


def _hi_lo(a):
    f8 = _f8()
    hi = a.astype(f8)
    lo = (a - hi.astype(np.float32)).astype(f8)
    return hi, lo


def _tile_x(xT8):
    """[1280, 1500] fp8 -> [p, ms, kp, two, j], cols zero-padded to 1536
    (s-tile-major DoubleRow pair layout)."""
    pad = np.zeros((D, KS * 128), dtype=xT8.dtype)
    pad[:, 0:S] = xT8
    a = pad.reshape(KD // 2, 2, 128, KS, 128)
    return np.ascontiguousarray(a.transpose(2, 3, 0, 1, 4))


def _tile_w8(W8):
    """[1280, 640] fp8 -> [m, p, kp, two, j]: each m-slice is a contiguous
    1.25KB/partition DMA in DoubleRow pair layout."""
    a = W8.reshape(KD // 2, 2, 128, MD, 128)
    return np.ascontiguousarray(a.transpose(3, 2, 0, 1, 4))


def _prep_in_maps(x, Wq, bq, Wk, Wv, Wo):
    in_maps = []
    for c in range(N_CORES):
        b, g = divmod(c, G)
        gs = slice(g * DG, (g + 1) * DG)
        xh, xl = _hi_lo(np.ascontiguousarray(x[b].T))
        im = {
            "xh": _tile_x(xh),
            "xl": _tile_x(xl),
            "wo": np.ascontiguousarray(Wo[:, gs].T).astype(np.float16),
            "bq": np.ascontiguousarray(
                (0.125 * bq[gs]).astype(np.float32).reshape(MD, 128).T),
        }
        for nm, W in (("q", Wq), ("k", Wk), ("v", Wv)):
            wh, wl = _hi_lo(np.ascontiguousarray(W[gs, :].T) * 32.0)
            im["w" + nm + "h"] = _tile_w8(wh)
            im["w" + nm + "l"] = _tile_w8(wl)
        in_maps.append(im)
    return in_maps


def run(x, Wq, bq, Wk, Wv, bv, Wo, bo, trace=False, **trace_kw):
    x = np.asarray(x, dtype=np.float32)
    Wq = np.asarray(Wq, dtype=np.float32)
    bq = np.asarray(bq, dtype=np.float32)
    Wk = np.asarray(Wk, dtype=np.float32)
    Wv = np.asarray(Wv, dtype=np.float32)
    bv = np.asarray(bv, dtype=np.float32)
    Wo = np.asarray(Wo, dtype=np.float32)
    bo = np.asarray(bo, dtype=np.float32)

    nc = _get_nc()
    in_maps = _prep_in_maps(x, Wq, bq, Wk, Wv, Wo)
    res = None
    for attempt in range(3):
        try:
            res = run_bass_kernel_spmd(nc, in_maps, list(range(N_CORES)),
                                       trace=trace, **trace_kw)
            break
        except Exception:
            # Sporadic NRT_EXEC_UNIT_UNRECOVERABLE on first exec; devices
            # come back after ~75s. Reset the backend and retry.
            if attempt == 2:
                raise
            import time as _time
            import jax as _jax
            _time.sleep(80)
            try:
                _jax.clear_backends()
            except Exception:
                pass
    const = (bv @ Wo.T + bo).astype(np.float32)  # [D]
    out = np.empty((B, S, D), dtype=np.float32)
    for b in range(B):
        out[b] = (res.results[2 * b]["out"].astype(np.float32)
                  + res.results[2 * b + 1]["out"].astype(np.float32) + const)
    return out, res


def kernel(**inputs):
    out, _ = run(**inputs)
    return out


# revision 36
# speedup vs baseline: 1.0955x; 1.0251x over previous
"""Trainium2 Bass kernel: Whisper-style self-attention (B=4, S=1500, D=1280, H=20).

Sharding: core c = 2*b + g handles batch b (of 4) and head-group g (of 2,
10 heads each).  Every matmul is exactly 1/8 of the total work:
  - Q/K/V projections column-sharded over the head group,
  - attention sharded by (batch, head),
  - output projection row-sharded; the two head-group partials of each batch
    are summed on the host (plus bias terms, which fold into host math).

Device dataflow (per core), all fp16 operands (PSUM f32):
  xT [1280,1500] -> qT,kT [640,1500] fp16 (qT scaled 1/8 + bq),
  v [1500,10,65] (64 v cols + ones col per head -> softmax Z).
  Per (head h, sq chunk c): scoresT = kT.T@qT per (128-row k tile), Exp
  batched over psum bank pairs on ACT -> expT fp16.  Then per 128-col sq
  subtile: ctx[sq,65] accumulated in PSUM with ex as the STATIONARY operand
  (12 matmuls of only 65 moving cols each - 2x fewer PE cycles than
  streaming expT), DVE reciprocal of the Z column + per-partition
  tensor_scalar multiply -> ctx_sb fp16 [sq,128] (head pair), then a DMA
  transpose (xbar) writes ctxT [128,sq] directly - no PE transpose, no
  PSUM->SBUF copy.  O-proj fp16 (ctxT stationary, wo moving).

Scheduling: sequencers are in-order and sem waits hold the SEQ, so the
emission order IS the schedule.  Attention units (software-pipelined one
unit deep) are ACT-bound: per 512-col chunk ACT needs ~6.2us of exp while
its scores+attnV PE work is only ~3.9us, and the ps2 WAR (bufs=2) stalls
PE inside the scores loop ~0.6us per psum pair.  So ALL other PE work
(projections, O-proj) is chopped into one-PSUM-group "micro" pieces and a
credit scheduler pumps ~one micro per scores pair, subject to
read-after-write deadlines (dl) and transpose-gating (nb).  ctx transposes
own the SP DMA queue; out stores ride Pool/SWDGE; weight loads are split
per 128-col slice so the first matmuls start ~2us after launch.
"""
import sys
sys.path.insert(0, "/opt/trn_rl_repo")

from collections import deque
from contextlib import ExitStack
import numpy as np

import concourse.bass as bass
import concourse.tile as tile
from concourse import bacc, mybir
from concourse.bass_utils import run_bass_kernel_spmd

dt = mybir.dt
AF = mybir.ActivationFunctionType
ALU = mybir.AluOpType

N_CORES = 8
B, S, D = 4, 1500, 1280
H, DH = 20, 64
G = 2
DG = D // G           # 640
HPG = H // G          # 10
KD = D // 128         # 10
MD = DG // 128        # 5
CW = (512, 512, 476)  # sq/proj chunk widths (PSUM-bank bound)
CO = (0, 512, 1024)   # chunk offsets
NS = 3
KS = (S + 127) // 128  # 12 (11*128 + 92)
ON = (512, 512, 256)
OO = (0, 512, 1024)
SP = S + 4            # ctxT padded to 1504 so the last 96-wide sq subtile
                      # (92 real cols) can be DMA-transposed whole

_CACHE = {}


def _sk(i):
    return min(128, S - i * 128)


def _subtiles(c):
    """(local_off, width) 128-col subtiles of chunk c; last one padded to 96."""
    w = CW[c]
    out = []
    off = 0
    while off < w:
        sw = min(128, w - off)
        if sw % 16:
            sw = 96  # pad 92 -> 96 for the xbar transpose (junk cols unread)
        out.append((off, sw))
        off += 128
    return out


def build():
    nc = bacc.Bacc("TRN2", target_bir_lowering=False, debug=False,
                   num_devices=N_CORES)
    # x and the QKV weights arrive as fp8e4m3 hi/lo pairs (W pre-scaled by
    # 32 so the lo residual stays out of fp8's subnormal range; descale is
    # folded into the psum->SBUF copy).  The projections then run DoubleRow
    # fp8 matmuls: K=256 per instruction at 0.5 cycles/row -> 4x the fp16
    # GEMM rate, 3 hi/lo terms (xh*wh + xh*wl + xl*wh) -> net 25% fewer PE
    # rows with ~0.1% relative error.  Layouts are pre-tiled host-side so
    # every DMA is a contiguous >=512B-row read.
    # x is staged s-tile-major [p, ms, kp, two, j] (cols zero-padded to
    # 1536): the v projection needs x STATIONARY with pair-adjacent blocks
    # (the ldweights ISA rejects a 1500-stride pair dim), and the qk
    # projections read the same tile as a nested moving AP.
    xh_d = nc.dram_tensor("xh", [128, KS, KD // 2, 2, 128], dt.float8e4,
                          kind="ExternalInput").ap()
    xl_d = nc.dram_tensor("xl", [128, KS, KD // 2, 2, 128], dt.float8e4,
                          kind="ExternalInput").ap()
    w8_d = {}
    for t8 in ("qh", "ql", "kh", "kl", "vh", "vl"):
        w8_d[t8] = nc.dram_tensor(
            "w" + t8, [MD, 128, KD // 2, 2, 128], dt.float8e4,
            kind="ExternalInput").ap()
    wo_d = nc.dram_tensor("wo", [DG, D], dt.float16, kind="ExternalInput").ap()
    bq_d = nc.dram_tensor("bq", [128, MD], dt.float32, kind="ExternalInput").ap()
    out_d = nc.dram_tensor("out", [S, D], dt.float16, kind="ExternalOutput").ap()

    wo_r = wo_d.rearrange("(k p) n -> p k n", p=128)
    KP = KD // 2  # 5 DoubleRow k-pair groups

    with tile.TileContext(nc) as tc, ExitStack() as octx:
        persist = octx.enter_context(tc.tile_pool(name="persist", bufs=1))
        epool = octx.enter_context(tc.tile_pool(name="expT", bufs=3))
        zpool = octx.enter_context(tc.tile_pool(name="z", bufs=3))
        cpool = octx.enter_context(tc.tile_pool(name="ctxsb", bufs=12))
        opool = octx.enter_context(tc.tile_pool(name="ob", bufs=3))
        ps2 = octx.enter_context(tc.tile_pool(name="ps2", bufs=2, space="PSUM"))
        ps1 = octx.enter_context(tc.tile_pool(name="ps1", bufs=2, space="PSUM"))
        pat = octx.enter_context(tc.tile_pool(name="pat", bufs=2, space="PSUM"))

        qT = persist.tile([128, MD, KS * 128], dt.float16, tag="qT")
        kT = persist.tile([128, MD, KS * 128], dt.float16, tag="kT")
        v = persist.tile([128, KS, HPG, DH + 1], dt.float16, tag="v")
        ctxT = persist.tile([128, MD, SP], dt.float16, tag="ctxT")
        bq_s = persist.tile([128, MD], dt.float32, tag="bq")
        xh_s = persist.tile([128, KS, KP, 2, 128], dt.float8e4, tag="xh")
        xl_s = persist.tile([128, KS, KP, 2, 128], dt.float8e4, tag="xl")
        w8 = {t8: persist.tile([128, MD, KP, 2, 128], dt.float8e4,
                               tag="w" + t8, name="w8")
              for t8 in ("qh", "ql", "kh", "kl", "vh", "vl")}
        wo_s = persist.tile([128, MD, D], dt.float16, tag="wo")

        # --- input DMAs, ordered for earliest first matmul (the cost model
        # serializes transfers on one DMA_ENGINES slot, so order matters):
        # the prelude computes kT c0, qT c0, v[h0-1] ms0-3, kT c1, v ms4-7,
        # kT c2 -- each group's data lands just before PE reaches it.
        def wdma(t8, m):
            nc.sync.dma_start(
                out=w8[t8][:, m].rearrange("p a b c -> p (a b c)"),
                in_=w8_d[t8][m].rearrange("p a b c -> p (a b c)"))

        def xdma(xs_s, xs_d, n):
            nc.sync.dma_start(
                out=xs_s[:, 4 * n:4 * n + 4].rearrange(
                    "p a b c d -> p (a b c d)"),
                in_=xs_d[:, 4 * n:4 * n + 4].rearrange(
                    "p a b c d -> p (a b c d)"))

        wdma("kh", 0)
        xdma(xh_s, xh_d, 0)
        wdma("kl", 0)
        xdma(xl_s, xl_d, 0)
        wdma("qh", 0)
        wdma("ql", 0)
        nc.sync.dma_start(out=bq_s[:], in_=bq_d[:])
        xdma(xh_s, xh_d, 1)
        xdma(xl_s, xl_d, 1)
        wdma("vh", 0)
        wdma("vl", 0)
        xdma(xh_s, xh_d, 2)
        xdma(xl_s, xl_d, 2)
        for m in range(1, MD):
            for t8 in ("kh", "kl", "qh", "ql", "vh", "vl"):
                wdma(t8, m)

        ones1 = persist.tile([128, 1], dt.float16, tag="ones1")
        nc.vector.memset(ones1[:], 1.0)
        nc.vector.tensor_copy(v[:, :, :, DH:DH + 1],
                              ones1[:].to_broadcast([128, KS, HPG, 1]))

        # ---- micro building blocks (one ps1 PSUM group each) -----------
        DR = mybir.MatmulPerfMode.DoubleRow
        TERMS = lambda t: ((xh_s, w8[t + "h"]), (xh_s, w8[t + "l"]),
                           (xl_s, w8[t + "h"]))

        def qk_micro(m, which, n):
            """One 512-col sq chunk of the q or k projection for d-tile m
            (~1.6us): 3-term hi/lo fp8 DoubleRow, descale 1/32 in the psum
            copy.  Chunk c2 includes the zero-padded cols 1500-1535."""
            co = CO[n]
            ps = ps1.tile([128, 1, 512], dt.float32, tag="ps1", name="ps1")
            first, last = (0, 0), (2, KP - 1)
            for ti, (xs, wt) in enumerate(TERMS(which)):
                for kp in range(KP):
                    nc.tensor.matmul(
                        ps[:, 0, 0:512],
                        lhsT=wt[:, m, kp],
                        rhs=xs[:, 4 * n:4 * n + 4, kp].rearrange(
                            "p a b c -> p b a c"),
                        start=((ti, kp) == first), stop=((ti, kp) == last),
                        perf_mode=DR)
            if which == "q":
                nc.vector.tensor_scalar(
                    qT[:, m, co:co + 512], ps[:, 0, 0:512], 0.125 / 32.0,
                    bq_s[:, m:m + 1], op0=ALU.mult, op1=ALU.add)
            else:
                nc.vector.tensor_scalar(
                    kT[:, m, co:co + 512], ps[:, 0, 0:512], 1.0 / 32.0,
                    None, op0=ALU.mult)

        def v_micro(hp, ms):
            """v columns for head pair hp, one 128-row s tile (~0.4us)."""
            sp = _sk(ms)
            ps = ps1.tile([128, 1, 512], dt.float32, tag="ps1", name="ps1")
            first, last = (0, 0), (2, KP - 1)
            for ti, (xs, wt) in enumerate(TERMS("v")):
                for kp in range(KP):
                    nc.tensor.matmul(
                        ps[0:sp, 0, 0:128],
                        lhsT=xs[:, ms, kp, :, 0:sp],
                        rhs=wt[:, hp, kp],
                        start=((ti, kp) == first), stop=((ti, kp) == last),
                        perf_mode=DR)
            nc.vector.tensor_scalar(
                v[0:sp, ms, 2 * hp:2 * hp + 2, 0:DH],
                ps[0:sp, 0, 0:128].rearrange("p (h e) -> p h e", h=2),
                1.0 / 32.0, None, op0=ALU.mult)

        def wo_micro():
            nc.sync.dma_start(out=wo_s[:], in_=wo_r[:])

        def op_micro(ms, j):
            """One 512-col group of the O-projection for sq tile ms."""
            sp = _sk(ms)
            nw, noff = ON[j], OO[j]
            if ms >= 8 and j % 2 == 1:
                # endgame: scores are done, borrow a ps2 bank to double the
                # psum rotation depth of the final O-proj chain
                ps = ps2.tile([128, 2, 512], dt.float32, tag="ps2",
                              name="ps2")[:, 0:1]
            else:
                ps = ps1.tile([128, 1, 512], dt.float32, tag="ps1",
                              name="ps1")
            for kk in range(MD):
                nc.tensor.matmul(
                    ps[0:sp, 0, 0:nw],
                    lhsT=ctxT[:, kk, ms * 128:ms * 128 + sp],
                    rhs=wo_s[:, kk, noff:noff + nw],
                    start=(kk == 0), stop=(kk == MD - 1))
            ob = opool.tile([128, 512], dt.float16, tag="ob", name="ob")
            if ms >= 8:
                # ACT is idle at the end while DVE still drains the last
                # tail's recip/mult chain
                nc.scalar.activation(ob[0:sp, 0:nw], ps[0:sp, 0, 0:nw],
                                     AF.Copy)
            else:
                nc.vector.tensor_copy(ob[0:sp, 0:nw], ps[0:sp, 0, 0:nw])
            # Mid-kernel out-stores ride Pool/SWDGE (SP.SEQ is busy with ctx
            # transposes whose sem waits hold it); the final sq tiles
            # alternate queues so the drain overlaps.
            eng = nc.sync if (ms >= 8 and j >= 1) else nc.gpsimd
            eng.dma_start(
                out=out_d[ms * 128:ms * 128 + sp, noff:noff + nw],
                in_=ob[0:sp, 0:nw])

        # ---- attention unit pieces -------------------------------------
        def emit_scores(h, c, pump):
            base = 64 * (h % 2)
            td = h // 2
            cw, co = CW[c], CO[c]
            csl = slice(co, co + cw)
            ex = epool.tile([128, KS, 512], dt.float16, tag="expT", name="ex")
            for kk2 in range(0, KS, 2):
                ps = ps2.tile([128, 2, 512], dt.float32, tag="ps2", name="ps2")
                for j in range(2):
                    kk = kk2 + j
                    sp = _sk(kk)
                    nc.tensor.matmul(
                        ps[0:sp, j, 0:cw],
                        lhsT=kT[base:base + 64, td, kk * 128:kk * 128 + sp],
                        rhs=qT[base:base + 64, td, csl],
                        start=True, stop=True)
                nc.scalar.activation(ex[:, kk2:kk2 + 2, 0:cw], ps[:, :, 0:cw],
                                     AF.Exp)
                pump()
            return ex

        csb_live = {}

        def emit_tail(h, c, ex, after_subtile=None):
            """attnV (ex stationary) + 1/Z scale into the pair's ctx_sb.
            For odd h the subtile's transpose is emitted as soon as both
            halves are written; after_subtile(t_idx) can interleave extra PE
            work (used to overlap the final O-proj with the last tail)."""
            td, hb = h // 2, 64 * (h % 2)
            if (td, c) not in csb_live:
                csb_live[(td, c)] = {
                    off: cpool.tile([128, 128], dt.float16, tag="ctxsb",
                                    name="ctxsb")
                    for off, _ in _subtiles(c)}
            csb = csb_live[(td, c)]
            for ti, (off, sw) in enumerate(_subtiles(c)):
                pc = pat.tile([128, DH + 1], dt.float32, tag="pat", name="pat")
                for kk in range(KS):
                    sp = _sk(kk)
                    nc.tensor.matmul(
                        pc[0:sw, :],
                        lhsT=ex[0:sp, kk, off:off + sw],
                        rhs=v[0:sp, kk, h, :],
                        start=(kk == 0), stop=(kk == KS - 1))
                rz = zpool.tile([128, 1], dt.float32, tag="rz", name="rz")
                nc.vector.reciprocal(rz[0:sw, :], pc[0:sw, DH:DH + 1])
                nc.vector.tensor_scalar(
                    csb[off][0:sw, hb:hb + 64], pc[0:sw, 0:DH], rz[0:sw, :],
                    None, op0=ALU.mult)
                if h % 2 == 1:
                    nc.sync.dma_start(
                        out=ctxT[:, td, CO[c] + off:CO[c] + off + sw],
                        in_=csb[off][0:sw, :], transpose=True)
                    if after_subtile is not None:
                        after_subtile(ti)
            if h % 2 == 1:
                del csb_live[(td, c)]

        # ---- schedule --------------------------------------------------
        # td-major pair order spreads the projection deadlines over the
        # first 18 iterations (c-major locks them all before iteration 10,
        # leaving the c1/c2 regions structurally short of PE filler).
        pairs = [(td, c) for td in range(5) for c in (0, 1)]
        pairs += [(td, 2) for td in range(5)]
        units = [(2 * td + o, c) for td, c in pairs for o in (0, 1)]

        # micro list: (cost_rows, dl, nb, fn); consumed strictly in order.
        # dl: must be emitted before scores of that iteration (RAW via
        # emission order).  nb: not before that iteration (transpose gating).
        M = []
        M.append((3840, 2, 0, lambda: qk_micro(0, "q", 1)))
        for m in range(1, MD):
            for n in range(NS):
                M.append((3840, 4 * m, 0,
                          lambda m=m, n=n: qk_micro(m, "k", n)))
            M.append((3840, 4 * m, 0, lambda m=m: qk_micro(m, "q", 0)))
            for ms in range(KS):
                M.append((960, 4 * m + 1, 0,
                          lambda m=m, ms=ms: v_micro(m, ms)))
            M.append((3840, 4 * m + 2, 0, lambda m=m: qk_micro(m, "q", 1)))
        M.append((0, 18, 0, wo_micro))
        # O-proj micros: nb-gated on their chunk's last transposes, dl
        # staggered so forced drains spread them over the late iterations.
        op_dl = {0: (19, 19, 20), 1: (20, 21, 21), 2: (22, 22, 23),
                 3: (23, 24, 24), 4: (21, 22, 23), 5: (24, 24, 25),
                 6: (25, 26, 26), 7: (27, 27, 28)}
        for ms in (0, 1):
            for j in range(NS):
                M.append((5 * ON[j], op_dl[ms][j], 19,
                          lambda ms=ms, j=j: op_micro(ms, j)))
        M.append((3840, 20, 18, lambda: qk_micro(0, "q", 2)))
        for ms in (2, 3):
            for j in range(NS):
                M.append((5 * ON[j], op_dl[ms][j], 19,
                          lambda ms=ms, j=j: op_micro(ms, j)))
        M.append((3840, 22, 20, lambda: qk_micro(1, "q", 2)))
        for ms in (4, 5):
            for j in range(NS):
                M.append((5 * ON[j], op_dl[ms][j], 21,
                          lambda ms=ms, j=j: op_micro(ms, j)))
        M.append((3840, 24, 22, lambda: qk_micro(2, "q", 2)))
        for ms in (6, 7):
            for j in range(NS):
                M.append((5 * ON[j], op_dl[ms][j], 21,
                          lambda ms=ms, j=j: op_micro(ms, j)))
        M.append((3840, 26, 24, lambda: qk_micro(3, "q", 2)))
        M.append((3840, 28, 26, lambda: qk_micro(4, "q", 2)))
        mq = deque(M)
        # pace matches the per-pair ACT deficit (~930 PE rows): ACT needs
        # ~6.2us/unit of exp vs ~3.9us of scores+attnV PE work.  Pumping
        # faster than the deficit just drains the queue early and leaves
        # the late units dry; deadline drains place the surplus.
        pace = 930.0

        state = {"iter": 0, "debt": 0.0}

        def drain_deadlines():
            # pop through the LAST due micro (due ones may sit behind
            # not-yet-due ops in the strictly-ordered queue)
            it = state["iter"]
            idx = -1
            for k, m in enumerate(mq):
                if m[1] <= it:
                    idx = k
            for _ in range(idx + 1):
                _, _, nb, fn = mq.popleft()
                assert nb <= it, "nb violation forced by a deadline"
                fn()

        def pump():
            state["debt"] += pace
            while mq and state["debt"] > 0 and mq[0][2] <= state["iter"]:
                cost, _, _, fn = mq.popleft()
                fn()
                state["debt"] -= cost

        # prelude: only kT/qT d-tile 0 chunk 0 -- unit 0's scores pairs are
        # then interleaved with the REST of the prelude (kT c1/c2, v[h0-1])
        # so attention starts ~6us earlier and the later kT chunk groups
        # hide the xt c1/c2 DMA waits behind ready scores/v work.
        qk_micro(0, "k", 0)
        qk_micro(0, "q", 0)
        p0seq = {
            1: [lambda: [v_micro(0, ms) for ms in range(0, 4)],
                lambda: qk_micro(0, "k", 1)],
            3: [lambda: [v_micro(0, ms) for ms in range(4, 8)],
                lambda: qk_micro(0, "k", 2)],
            5: [lambda: [v_micro(0, ms) for ms in range(8, KS)]],
        }
        p0 = {"j": 0}

        def pump0():
            j = p0["j"]
            p0["j"] += 1
            for f in p0seq.get(j, []):
                f()

        exm = {}
        for i, u in enumerate(units):
            state["iter"] = i
            drain_deadlines()
            exm[u] = emit_scores(u[0], u[1], pump0 if i == 0 else pump)
            if i >= 1:
                up = units[i - 1]
                emit_tail(up[0], up[1], exm.pop(up))
        up = units[-1]
        state["iter"] = len(units)
        while mq:
            _, _, _, fn = mq.popleft()
            fn()

        def tail_hook(ti):
            # overlap the final O-proj with the last tail: two subtiles
            # after a transpose, its O-proj runs (the in-between attnV +
            # O-proj work hides the transpose DMA latency).
            if ti >= 2:
                for j in range(NS):
                    op_micro(8 + ti - 2, j)
        emit_tail(up[0], up[1], exm.pop(up), after_subtile=tail_hook)
        for ms in (10, 11):
            for j in range(NS):
                op_micro(ms, j)

    nc.compile()
    return nc


def _get_nc():
    if "nc" not in _CACHE:
        _CACHE["nc"] = build()
    return _CACHE["nc"]


F8 = None


def _f8():
    global F8
    if F8 is None:
        import ml_dtypes
        F8 = ml_dtypes.float8_e4m3
    return F8


def _hi_lo(a):
    f8 = _f8()
    hi = a.astype(f8)
    lo = (a - hi.astype(np.float32)).astype(f8)
    return hi, lo


def _tile_x(xT8):
    """[1280, 1500] fp8 -> [p, ms, kp, two, j], cols zero-padded to 1536
    (s-tile-major DoubleRow pair layout)."""
    pad = np.zeros((D, KS * 128), dtype=xT8.dtype)
    pad[:, 0:S] = xT8
    a = pad.reshape(KD // 2, 2, 128, KS, 128)
    return np.ascontiguousarray(a.transpose(2, 3, 0, 1, 4))


def _tile_w8(W8):
    """[1280, 640] fp8 -> [m, p, kp, two, j]: each m-slice is a contiguous
    1.25KB/partition DMA in DoubleRow pair layout."""
    a = W8.reshape(KD // 2, 2, 128, MD, 128)
    return np.ascontiguousarray(a.transpose(3, 2, 0, 1, 4))


def _prep_in_maps(x, Wq, bq, Wk, Wv, Wo):
    in_maps = []
    for c in range(N_CORES):
        b, g = divmod(c, G)
        gs = slice(g * DG, (g + 1) * DG)
        xh, xl = _hi_lo(np.ascontiguousarray(x[b].T))
        im = {
            "xh": _tile_x(xh),
            "xl": _tile_x(xl),
            "wo": np.ascontiguousarray(Wo[:, gs].T).astype(np.float16),
            "bq": np.ascontiguousarray(
                (0.125 * bq[gs]).astype(np.float32).reshape(MD, 128).T),
        }
        for nm, W in (("q", Wq), ("k", Wk), ("v", Wv)):
            wh, wl = _hi_lo(np.ascontiguousarray(W[gs, :].T) * 32.0)
            im["w" + nm + "h"] = _tile_w8(wh)
            im["w" + nm + "l"] = _tile_w8(wl)
        in_maps.append(im)
    return in_maps


def run(x, Wq, bq, Wk, Wv, bv, Wo, bo, trace=False, **trace_kw):
    x = np.asarray(x, dtype=np.float32)
    Wq = np.asarray(Wq, dtype=np.float32)
    bq = np.asarray(bq, dtype=np.float32)
    Wk = np.asarray(Wk, dtype=np.float32)
    Wv = np.asarray(Wv, dtype=np.float32)
    bv = np.asarray(bv, dtype=np.float32)
    Wo = np.asarray(Wo, dtype=np.float32)
    bo = np.asarray(bo, dtype=np.float32)

    nc = _get_nc()
    in_maps = _prep_in_maps(x, Wq, bq, Wk, Wv, Wo)
    res = None
    for attempt in range(3):
        try:
            res = run_bass_kernel_spmd(nc, in_maps, list(range(N_CORES)),
                                       trace=trace, **trace_kw)
            break
        except Exception:
            # Sporadic NRT_EXEC_UNIT_UNRECOVERABLE on first exec; devices
            # come back after ~75s. Reset the backend and retry.
            if attempt == 2:
                raise
            import time as _time
            import jax as _jax
            _time.sleep(80)
            try:
                _jax.clear_backends()
            except Exception:
                pass
    const = (bv @ Wo.T + bo).astype(np.float32)  # [D]
    out = np.empty((B, S, D), dtype=np.float32)
    for b in range(B):
        out[b] = (res.results[2 * b]["out"].astype(np.float32)
                  + res.results[2 * b + 1]["out"].astype(np.float32) + const)
    return out, res


def kernel(**inputs):
    out, _ = run(**inputs)
    return out
